# revision 1
# baseline (speedup 1.0000x reference)
"""DrBCNet GNN message-passing kernel for 8 Trainium2 NeuronCores — sparse v4.

Strategy (dst-sharded, sparse gather + selector matmuls):
  - Nodes globally degree-sorted, dealt round-robin to 8 cores (3750 each);
    GRU/norm/decoder node-local in column layout (hT [feat, pos]).
  - The h row-table (fp8, [30848, 128]) lives in DRAM (Shared), rebuilt each
    layer by two AllGather halves (pos<2048 | pos>=2048).
  - Aggregation: per layer each core dma_gathers h[src] rows for its ~75k
    in-edges (1024-idx SWDGE chunks; 256B descriptors), then accumulates
    aggT[feat, dst-col] strips in PSUM via small selector matmuls
    (lhsT = gathered edge tile [128e x 128f], rhs = 0/1 fp8 window).
  - Edge schedule is SPMD-uniform: shared window boundaries (<=512 edges per
    window on every core, 4 tiles each); per-core data = gather indices +
    fp8 selector entries.
  - GRU: bf16 weight lhsT (1 cyc/row), i+h gate sums accumulated in PSUM,
    biases folded into the PSUM-evacuating activations; Sqrt (l2norm) is
    phase-batched per half to avoid ACT table swaps.
"""

import functools
import os

import numpy as np

CORES = 8
H = 128
L = 5
BANK = 512
NORM_EPS_SQ = 1e-24
WIN_EDGES = int(os.environ.get("WE", "512"))
TPW = WIN_EDGES // 128
H0_POS = int(os.environ.get("H0", "3072"))
TBL_FP8 = os.environ.get("TBL", "fp8") == "fp8"


# ---------------------------------------------------------------- host planning
def _plan(edge_src, edge_dst, n_nodes):
    npc = n_nodes // CORES
    npc_pad = ((npc + 127) // 128) * 128
    ntiles = npc_pad // 128

    deg = np.bincount(edge_dst, minlength=n_nodes)
    gorder = np.argsort(-deg, kind="stable")
    gpos = np.empty(n_nodes, np.int64)
    gpos[gorder] = np.arange(n_nodes)
    owner = gpos % CORES
    pos = gpos // CORES
    order_per_core = [gorder[r::CORES] for r in range(CORES)]

    # table row: half-major so each AllGather half is a contiguous row range
    h1_pos = npc_pad - H0_POS
    half = (pos >= H0_POS).astype(np.int64)
    hsize = np.where(half == 1, h1_pos, H0_POS)
    hbase = half * (CORES * H0_POS)
    tpos = hbase + owner * hsize + (pos - half * H0_POS)

    srcrow_all = tpos[edge_src]

    # per-core per-column counts split by src parity (of table row id)
    n_banks = (npc + BANK - 1) // BANK
    par_all = srcrow_all_par = None
    Ce = np.zeros((CORES, npc), np.int64)
    Co = np.zeros((CORES, npc), np.int64)
    tpar = tpos[edge_src] % 2
    dpos = pos[edge_dst]
    down = owner[edge_dst]
    for r in range(CORES):
        m = down == r
        np.add.at(Ce[r], dpos[m & (tpar == 0)], 1)
        np.add.at(Co[r], dpos[m & (tpar == 1)], 1)
    HALF_W = WIN_EDGES // 2
    windows = []
    for b in range(n_banks):
        c0, c1 = b * BANK, min((b + 1) * BANK, npc)
        w0 = c0
        rune = np.zeros(CORES, np.int64)
        runo = np.zeros(CORES, np.int64)
        for q in range(c0, c1):
            if (rune + Ce[:, q]).max() > HALF_W or (runo + Co[:, q]).max() > HALF_W:
                windows.append((w0, q))
                w0 = q
                rune = Ce[:, q].copy()
                runo = Co[:, q].copy()
            else:
                rune += Ce[:, q]
                runo += Co[:, q]
        windows.append((w0, c1))
    nwin = len(windows)
    win_width = [b - a for a, b in windows]
    sel_off = np.concatenate([[0], np.cumsum([TPW * w for w in win_width])])
    total_sel = int(sel_off[-1])
    total_idx = nwin * WIN_EDGES

    import ml_dtypes

    # tiles 0-1 of each window: even-parity srcs; tiles 2-3: odd. idx is the
    # PAIR row (tpos//2) when TBL_FP8 else the row itself.
    idx_maps, sel_maps = [], []
    for r in range(CORES):
        eidx = np.nonzero(down == r)[0]
        dp = dpos[eidx]
        sr = srcrow_all[eidx]
        o = np.argsort(dp * 2 + (sr % 2), kind="stable")
        dp, sr = dp[o], sr[o]
        par = sr % 2
        idxs = np.zeros(total_idx, np.int16)
        sel = np.zeros((128, total_sel), np.float32)
        wstarts = np.array([a for a, _ in windows] + [npc]) * 2
        bounds_e = np.searchsorted(dp * 2 + par, wstarts)
        for w, (a, b) in enumerate(windows):
            lo, hi = bounds_e[w], bounds_e[w + 1]
            S = b - a
            base = w * WIN_EDGES
            seg = slice(lo, hi)
            pvals = par[seg]
            for pp in (0, 1):
                m = np.nonzero(pvals == pp)[0]
                cnt = len(m)
                assert cnt <= WIN_EDGES // 2, (r, w, pp, cnt)
                slot0 = base + pp * (WIN_EDGES // 2)
                rows_sr = sr[seg][m]
                if TBL_FP8:
                    idxs[slot0 : slot0 + cnt] = (rows_sr // 2).astype(np.int16)
                else:
                    idxs[slot0 : slot0 + cnt] = rows_sr.astype(np.int16)
                loc = (dp[seg][m] - a).astype(np.int64)
                j = np.arange(cnt) // 128 + (TPW // 2) * pp
                p = np.arange(cnt) % 128
                sel[p, sel_off[w] + j * S + loc] = 1.0
        idx_w = np.zeros((128, total_idx // 16), np.int16)
        wrapped = idxs.reshape(total_idx // 16, 16).T
        for g in range(8):
            idx_w[g * 16 : (g + 1) * 16, :] = wrapped
        idx_maps.append(idx_w)
        sel_maps.append(sel.astype(ml_dtypes.float8_e4m3fn))

    bank_wins = []
    for b in range(n_banks):
        ws = [w for w, (a, _) in enumerate(windows) if a // BANK == b]
        bank_wins.append((min(ws), max(ws)))

    return dict(
        tile_par=[0, 0, 1, 1],
        npc=npc,
        npc_pad=npc_pad,
        ntiles=ntiles,
        n_banks=n_banks,
        nwin=nwin,
        windows=windows,
        sel_off=sel_off,
        total_sel=total_sel,
        total_idx=total_idx,
        bank_wins=bank_wins,
        order_per_core=order_per_core,
        idx_maps=idx_maps,
        sel_maps=sel_maps,
    )


# ---------------------------------------------------------------- bass program
def _build(meta):
    import concourse.bacc as bacc
    import concourse.mybir as mybir
    import concourse.tile as tile
    from concourse import library_config
    from concourse.bass import AP
    from concourse.masks import make_identity

    npc = meta["npc"]
    npc_pad = meta["npc_pad"]
    ntiles = meta["ntiles"]
    n_banks = meta["n_banks"]
    windows = meta["windows"]
    sel_off = meta["sel_off"]
    total_sel = meta["total_sel"]
    total_idx = meta["total_idx"]
    bank_wins = meta["bank_wins"]
    n_tbl = CORES * npc_pad
    f32 = mybir.dt.float32
    bf16 = mybir.dt.bfloat16
    i16 = mybir.dt.int16
    fp8 = mybir.dt.float8e4
    tbl_dt = fp8 if TBL_FP8 else bf16
    AF = mybir.ActivationFunctionType
    OP = mybir.AluOpType

    h0_rows = CORES * H0_POS

    nc = bacc.Bacc(
        "TRN2",
        target_bir_lowering=False,
        debug=False,
        num_devices=CORES,
        dynamic_dma_scratch_size=int(os.environ.get("SCR", "32768")),
    )

    # I/O
    xT_d = nc.dram_tensor("xT", [3, npc], f32, kind="ExternalInput")
    idx_d = nc.dram_tensor("idx", [128, total_idx // 16], i16, kind="ExternalInput")
    sel_d = nc.dram_tensor("sel", [128, total_sel], fp8, kind="ExternalInput")
    w1T_d = nc.dram_tensor("w1T", [3, 128], f32, kind="ExternalInput")
    b1_d = nc.dram_tensor("b1", [128, 1], f32, kind="ExternalInput")
    wihT_d = nc.dram_tensor("wihT", [128, 3 * H], bf16, kind="ExternalInput")
    whhT_d = nc.dram_tensor("whhT", [128, 3 * H], bf16, kind="ExternalInput")
    brz_d = nc.dram_tensor("brz", [128, 4], f32, kind="ExternalInput")  # br,bz,bin,bhn
    w2T_d = nc.dram_tensor("w2T", [128, 128], f32, kind="ExternalInput")
    b2_d = nc.dram_tensor("b2", [1, 128], f32, kind="ExternalInput")
    binrow_d = nc.dram_tensor("binrow", [1, 128], bf16, kind="ExternalInput")
    bhnrow_d = nc.dram_tensor("bhnrow", [1, 128], bf16, kind="ExternalInput")
    out_d = nc.dram_tensor("out", [npc_pad, 128], f32, kind="ExternalOutput")

    if TBL_FP8:
        ag_in = [
            nc.dram_tensor(f"agin{l}", [npc_pad // 2, 2, 128], tbl_dt)
            for l in range(L)
        ]
        tables = [
            nc.dram_tensor(
                f"table{l}", [n_tbl // 2, 256], tbl_dt, addr_space="Shared"
            )
            for l in range(L)
        ]
    else:
        ag_in = [nc.dram_tensor(f"agin{l}", [npc_pad, 128], tbl_dt) for l in range(L)]
        tables = [
            nc.dram_tensor(f"table{l}", [n_tbl, 128], tbl_dt, addr_space="Shared")
            for l in range(L)
        ]
    groups = [list(range(CORES))]

    banks = [(b * BANK, min(BANK, npc - b * BANK)) for b in range(n_banks)]
    half_banks = [
        [b for b in range(n_banks) if banks[b][0] < H0_POS],
        [b for b in range(n_banks) if banks[b][0] >= H0_POS],
    ]

    with tile.TileContext(nc) as tc:
        import contextlib

        stack = contextlib.ExitStack()
        nc.gpsimd.load_library(library_config.mlp)
        per = stack.enter_context(tc.tile_pool(name="per", bufs=1))

        def _T(tcx, shape, dtype, name=None):
            return per.tile(shape, dtype, name=name, tag=name)

        xT_sb = _T(tc, [3, npc], f32, name="xT_sb")
        idx_sb = _T(tc, [128, total_idx // 16], i16, name="idx_sb")
        sel_sb = _T(tc, [128, total_sel], fp8, name="sel_sb")
        hT = _T(tc, [128, npc], f32, name="hT")
        hmaxT = _T(tc, [128, npc], f32, name="hmaxT")
        hT16 = _T(tc, [128, npc], bf16, name="hT16")
        w1T_sb = _T(tc, [3, 128], f32, name="w1T_sb")
        b1_sb = _T(tc, [128, 1], f32, name="b1_sb")
        wihT_sb = _T(tc, [128, 3 * H], bf16, name="wihT_sb")
        whhT_sb = _T(tc, [128, 3 * H], bf16, name="whhT_sb")
        brz_sb = _T(tc, [128, 4], f32, name="brz_sb")
        w2T_sb = _T(tc, [128, 128], f32, name="w2T_sb")
        b2_sb = _T(tc, [1, 128], f32, name="b2_sb")
        ones_col = _T(tc, [128, 1], f32, name="ones_col")
        ones_row = _T(tc, [1, BANK], bf16, name="ones_row")
        binrow = _T(tc, [1, 128], bf16, name="binrow")
        bhnrow = _T(tc, [1, 128], bf16, name="bhnrow")
        onesk1 = _T(tc, [1, 128], f32, name="onesk1")
        ident = _T(tc, [128, 128], f32, name="ident")
        eps_sb = _T(tc, [1, 1], f32, name="eps_sb")

        gpool = stack.enter_context(tc.tile_pool(name="gpool", bufs=int(os.environ.get("GB","4"))))
        epool = stack.enter_context(tc.tile_pool(name="epool", bufs=int(os.environ.get("EB","12"))))
        xpool = stack.enter_context(tc.tile_pool(name="xpool", bufs=2))
        tpool = stack.enter_context(tc.tile_pool(name="tpool", bufs=int(os.environ.get("TB","4"))))
        ps = stack.enter_context(tc.tile_pool(name="ps", bufs=8, space="PSUM"))

        nc.sync.dma_start(out=xT_sb[:], in_=xT_d[:])
        nc.sync.dma_start(out=idx_sb[:], in_=idx_d[:])
        nc.sync.dma_start(out=sel_sb[:], in_=sel_d[:])
        nc.sync.dma_start(out=w1T_sb[:], in_=w1T_d[:])
        nc.sync.dma_start(out=b1_sb[:], in_=b1_d[:])
        nc.sync.dma_start(out=wihT_sb[:], in_=wihT_d[:])
        nc.sync.dma_start(out=whhT_sb[:], in_=whhT_d[:])
        nc.sync.dma_start(out=brz_sb[:], in_=brz_d[:])
        nc.sync.dma_start(out=w2T_sb[:], in_=w2T_d[:])
        nc.sync.dma_start(out=b2_sb[:], in_=b2_d[:])
        nc.vector.memset(eps_sb[:], NORM_EPS_SQ)
        nc.vector.memset(ones_col[:], 1.0)
        nc.vector.memset(ones_row[:], 1.0)
        nc.sync.dma_start(out=binrow[:], in_=binrow_d[:])
        nc.sync.dma_start(out=bhnrow[:], in_=bhnrow_d[:])
        nc.vector.memset(onesk1[:], 1.0)
        make_identity(nc, ident[:])

        def norm_phase(bank_list):
            """l2norm hT strips for several banks; single Sqrt table window."""
            ns_list = []
            for b in bank_list:
                s0, w = banks[b]
                sq = tpool.tile([128, BANK], f32, tag="sq", name=f"sq{b}")
                nc.vector.tensor_tensor(
                    out=sq[:, :w], in0=hT[:, s0 : s0 + w], in1=hT[:, s0 : s0 + w],
                    op=OP.mult,
                )
                ns_ps = ps.tile([1, BANK], f32, tag="ps", name=f"ns{b}")
                nc.tensor.matmul(
                    out=ns_ps[:1, :w], lhsT=ones_col[:], rhs=sq[:, :w],
                    start=True, stop=True,
                )
                ns_list.append(ns_ps)
            inv_list = []
            for b, ns_ps in zip(bank_list, ns_list):
                s0, w = banks[b]
                srt = tpool.tile([1, BANK], f32, tag="srt", name=f"srt{b}")
                nc.scalar.activation(
                    out=srt[:1, :w], in_=ns_ps[:1, :w], func=AF.Sqrt,
                    bias=eps_sb[:1, :1],
                )
                inv_t = tpool.tile([1, BANK], f32, tag="inv_t", name=f"inv{b}")
                nc.vector.reciprocal(out=inv_t[:1, :w], in_=srt[:1, :w])
                inv_list.append(inv_t)
            for b, inv_t in zip(bank_list, inv_list):
                s0, w = banks[b]
                bc_ps = ps.tile([128, BANK], f32, tag="ps", name=f"bc{b}")
                nc.tensor.matmul(
                    out=bc_ps[:, :w], lhsT=onesk1[:1, :], rhs=inv_t[:1, :w],
                    start=True, stop=True,
                )
                nc.vector.tensor_tensor(
                    out=hT[:, s0 : s0 + w], in0=hT[:, s0 : s0 + w],
                    in1=bc_ps[:, :w], op=OP.mult,
                )
                nc.vector.tensor_copy(
                    out=hT16[:, s0 : s0 + w], in_=hT[:, s0 : s0 + w]
                )
                nc.vector.tensor_tensor(
                    out=hmaxT[:, s0 : s0 + w], in0=hmaxT[:, s0 : s0 + w],
                    in1=hT[:, s0 : s0 + w], op=OP.max,
                )

        def gru_bank(l, b, aggS):
            """GRU for bank b; agg strip in SBUF (aggS). Updates hT strip
            (pre-norm). ACT funcs used: Sigmoid/Copy/Tanh only."""
            s0, w = banks[b]
            # r and z: (W_ih x agg + W_hh x h) accumulated in PSUM
            rz = []
            for g in (0, 1):
                g_ps = ps.tile([128, BANK], f32, tag="ps", name=f"rz{l}{b}{g}")
                nc.tensor.matmul(
                    out=g_ps[:, :w], lhsT=wihT_sb[:, g * H : (g + 1) * H],
                    rhs=aggS[:, :w], start=True, stop=False,
                )
                nc.tensor.matmul(
                    out=g_ps[:, :w], lhsT=whhT_sb[:, g * H : (g + 1) * H],
                    rhs=hT16[:, s0 : s0 + w], start=False, stop=True,
                )
                gt = gpool.tile([128, BANK], f32, tag=f"g{g}", name=f"gs{l}{b}{g}")
                nc.scalar.activation(
                    out=gt[:, :w], in_=g_ps[:, :w], func=AF.Sigmoid,
                    bias=brz_sb[:, g : g + 1],
                )
                rz.append(gt)
            r_t, z_t = rz
            in_ps = ps.tile([128, BANK], f32, tag="ps", name=f"in{l}{b}")
            nc.tensor.matmul(
                out=in_ps[:, :w], lhsT=binrow[:1, :], rhs=ones_row[:1, :w],
                start=True, stop=False,
            )
            nc.tensor.matmul(
                out=in_ps[:, :w], lhsT=wihT_sb[:, 2 * H : 3 * H],
                rhs=aggS[:, :w], start=False, stop=True,
            )
            i_n = gpool.tile([128, BANK], f32, tag="gin", name=f"gin{l}{b}")
            nc.scalar.activation(out=i_n[:, :w], in_=in_ps[:, :w], func=AF.Copy)
            hn_ps = ps.tile([128, BANK], f32, tag="ps", name=f"hn{l}{b}")
            nc.tensor.matmul(
                out=hn_ps[:, :w], lhsT=bhnrow[:1, :], rhs=ones_row[:1, :w],
                start=True, stop=False,
            )
            nc.tensor.matmul(
                out=hn_ps[:, :w], lhsT=whhT_sb[:, 2 * H : 3 * H],
                rhs=hT16[:, s0 : s0 + w], start=False, stop=True,
            )
            h_n = gpool.tile([128, BANK], f32, tag="ghn", name=f"ghn{l}{b}")
            nc.scalar.activation(out=h_n[:, :w], in_=hn_ps[:, :w], func=AF.Copy)
            n_t = tpool.tile([128, BANK], f32, tag="n_t", name=f"n{l}{b}")
            nc.vector.tensor_tensor(
                out=n_t[:, :w], in0=r_t[:, :w], in1=h_n[:, :w], op=OP.mult
            )
            nc.vector.tensor_tensor(
                out=n_t[:, :w], in0=n_t[:, :w], in1=i_n[:, :w], op=OP.add
            )
            nc.scalar.activation(out=n_t[:, :w], in_=n_t[:, :w], func=AF.Tanh)
            d_t = tpool.tile([128, BANK], f32, tag="d_t", name=f"d{l}{b}")
            nc.vector.tensor_tensor(
                out=d_t[:, :w], in0=hT[:, s0 : s0 + w], in1=n_t[:, :w],
                op=OP.subtract,
            )
            nc.vector.tensor_tensor(
                out=d_t[:, :w], in0=d_t[:, :w], in1=z_t[:, :w], op=OP.mult
            )
            nc.vector.tensor_tensor(
                out=hT[:, s0 : s0 + w], in0=d_t[:, :w], in1=n_t[:, :w], op=OP.add
            )

        def store_half_and_allgather(l, half):
            t0 = 0 if half == 0 else H0_POS // 128
            t1 = H0_POS // 128 if half == 0 else ntiles
            nt_h = t1 - t0
            maxnt = max(H0_POS // 128, ntiles - H0_POS // 128)
            rows = xpool.tile(
                [128, maxnt, 128], tbl_dt, tag="xbuf", name=f"rows{l}_{half}"
            )
            if half == 1 and npc < npc_pad:
                nc.vector.memset(rows[:, nt_h - 1, :], 0.0)
            for ti in range(t0, t1):
                wt = min(128, npc - ti * 128)
                if wt <= 0:
                    break
                tp_ps = ps.tile([128, 128], f32, tag="ps", name=f"tp{l}_{ti}")
                nc.tensor.transpose(
                    out=tp_ps[:wt, :], in_=hT[:, ti * 128 : ti * 128 + wt],
                    identity=ident[:],
                )
                nc.scalar.activation(
                    out=rows[:wt, ti - t0, :], in_=tp_ps[:wt, :], func=AF.Copy
                )
            p0, p1 = (0, H0_POS) if half == 0 else (H0_POS, npc_pad)
            if TBL_FP8:
                dst = (
                    ag_in[l]
                    .ap()[p0 // 2 : p1 // 2, :, :]
                    .rearrange("(t k) q f -> (k q) t f", k=64)
                )
            else:
                dst = ag_in[l].ap()[p0:p1, :].rearrange("(c p) f -> p c f", p=128)
            nc.sync.dma_start(out=dst, in_=rows[:, :nt_h, :])
            if os.environ.get("SKIP_AG"):
                return
            r0 = 0 if half == 0 else h0_rows
            r1 = h0_rows if half == 0 else n_tbl
            if TBL_FP8:
                ins_ap = ag_in[l].ap()[p0 // 2 : p1 // 2, :, :]
                outs_ap = tables[l].ap()[r0 // 2 : r1 // 2, :]
            else:
                ins_ap = ag_in[l].ap()[p0:p1, :]
                outs_ap = tables[l].ap()[r0:r1, :]
            nc.gpsimd.collective_compute(
                "AllGather",
                mybir.AluOpType.bypass,
                replica_groups=groups,
                ins=[ins_ap],
                outs=[outs_ap],
            )

        # ---------------- encoder
        enc_banks = list(range(n_banks))
        for b, (s0, w) in enumerate(banks):
            h0_ps = ps.tile([128, BANK], f32, tag="ps", name=f"enc{b}")
            nc.tensor.matmul(
                out=h0_ps[:, :w], lhsT=w1T_sb[:], rhs=xT_sb[:, s0 : s0 + w],
                start=True, stop=True,
            )
            nc.scalar.activation(
                out=hT[:, s0 : s0 + w], in_=h0_ps[:, :w], func=AF.Relu,
                bias=b1_sb[:, :1],
            )
            nc.vector.memset(hmaxT[:, s0 : s0 + w], -1e30)
        norm_phase(enc_banks)
        store_half_and_allgather(0, 0)
        store_half_and_allgather(0, 1)

        # ---------------- message-passing layers
        for l in range(L):
            src_ap = tables[l][:]
            for half in (0, 1):
                hw0 = bank_wins[half_banks[half][0]][0]
                hw1 = bank_wins[half_banks[half][-1]][1]
                chunk_of = {}
                CPW = max(1, 1024 // WIN_EDGES)
                for w in range(hw0, hw1 + 1, CPW):
                    wlast = min(w + CPW - 1, hw1)
                    nidx = (wlast - w + 1) * WIN_EDGES
                    ew = 256 if TBL_FP8 else H
                    g_sb = epool.tile(
                        [128, 8, ew], tbl_dt, tag="gbuf", name=f"g{l}_{w}"
                    )
                    nc.gpsimd.dma_gather(
                        g_sb[:, : nidx // 128, :],
                        src_ap,
                        idx_sb[:, w * WIN_EDGES // 16 : (wlast + 1) * WIN_EDGES // 16],
                        nidx,
                        nidx,
                        ew,
                    )
                    for ww in range(w, wlast + 1):
                        chunk_of[ww] = (g_sb, (ww - w) * TPW)
                for b in half_banks[half]:
                    s0, wb = banks[b]
                    apb = ps.tile([128, BANK], f32, tag="ps", name=f"agg{l}_{b}")
                    w_first, w_last = bank_wins[b]
                    for w in range(w_first, w_last + 1):
                        a, e = windows[w]
                        S = e - a
                        g_sb, slot0 = chunk_of[w]
                        for j in range(TPW):
                            if TBL_FP8:
                                pp = j // (TPW // 2)
                                lt = g_sb[:, slot0 + j, pp * 128 : (pp + 1) * 128]
                            else:
                                lt = g_sb[:, slot0 + j, :]
                            nc.tensor.matmul(
                                out=apb[:, a - s0 : a - s0 + S],
                                lhsT=lt,
                                rhs=sel_sb[
                                    :, sel_off[w] + j * S : sel_off[w] + (j + 1) * S
                                ],
                                start=(j == 0),
                                stop=(j == TPW - 1),
                            )
                    aggS = gpool.tile([128, BANK], bf16, tag="aggS", name=f"as{l}{b}")
                    nc.scalar.activation(
                        out=aggS[:, :wb], in_=apb[:, :wb], func=AF.Copy
                    )
                    gru_bank(l, b, aggS)
                norm_phase(half_banks[half])
                if l < L - 1:
                    store_half_and_allgather(l + 1, half)

        # ---------------- decoder
        for t in range(ntiles):
            wt = min(128, npc - t * 128)
            o_ps = ps.tile([128, 128], f32, tag="ps", name=f"dec{t}")
            nc.tensor.matmul(
                out=o_ps[:wt, :], lhsT=onesk1[:1, :wt], rhs=b2_sb[:1, :],
                start=True, stop=False,
            )
            nc.tensor.matmul(
                out=o_ps[:wt, :], lhsT=hmaxT[:, t * 128 : t * 128 + wt],
                rhs=w2T_sb[:], start=False, stop=True,
            )
            orow = tpool.tile([128, 128], f32, tag="orow", name=f"or{t}")
            nc.scalar.activation(out=orow[:wt, :], in_=o_ps[:wt, :], func=AF.Copy)
            nc.sync.dma_start(
                out=out_d[t * 128 : t * 128 + wt, :], in_=orow[:wt, :]
            )
        stack.close()

    nc.compile()
    return nc


# ---------------------------------------------------------------- entry points
def _prep(inputs):
    import ml_dtypes

    x = np.asarray(inputs["x"], np.float32)
    edge_src = np.asarray(inputs["edge_src"], np.int64)
    edge_dst = np.asarray(inputs["edge_dst"], np.int64)
    n_nodes = x.shape[0]
    meta = _plan(edge_src, edge_dst, n_nodes)

    W1 = np.asarray(inputs["W1"], np.float32)
    b1 = np.asarray(inputs["b1"], np.float32)
    W_ih = np.asarray(inputs["W_ih"], np.float32)
    b_ih = np.asarray(inputs["b_ih"], np.float32)
    W_hh = np.asarray(inputs["W_hh"], np.float32)
    b_hh = np.asarray(inputs["b_hh"], np.float32)
    W2 = np.asarray(inputs["W2"], np.float32)
    b2 = np.asarray(inputs["b2"], np.float32)

    brz = np.stack(
        [
            b_ih[0:128] + b_hh[0:128],
            b_ih[128:256] + b_hh[128:256],
            b_ih[256:384],
            b_hh[256:384],
        ],
        axis=1,
    ).astype(np.float32)

    shared = dict(
        w1T=np.ascontiguousarray(W1.T),
        b1=np.ascontiguousarray(b1[:, None]),
        wihT=np.ascontiguousarray(W_ih.T).astype(ml_dtypes.bfloat16),
        whhT=np.ascontiguousarray(W_hh.T).astype(ml_dtypes.bfloat16),
        brz=brz,
        w2T=np.ascontiguousarray(W2.T),
        binrow=np.ascontiguousarray(b_ih[256:384][None, :]).astype(ml_dtypes.bfloat16),
        bhnrow=np.ascontiguousarray(b_hh[256:384][None, :]).astype(ml_dtypes.bfloat16),
        b2=np.ascontiguousarray(b2[None, :]),
    )
    in_maps = []
    for r in range(CORES):
        xr = x[meta["order_per_core"][r]]
        in_maps.append(
            dict(
                xT=np.ascontiguousarray(xr.T),
                idx=meta["idx_maps"][r],
                sel=meta["sel_maps"][r],
                **shared,
            )
        )
    return meta, in_maps


def _assemble(meta, results, n_nodes):
    npc = meta["npc"]
    out = np.empty((n_nodes, 128), np.float32)
    for r in range(CORES):
        out[meta["order_per_core"][r]] = results[r]["out"][:npc]
    return out


@functools.lru_cache(maxsize=1)
def _get_compiled(key):
    meta, in_maps = _PENDING[key]
    nc = _build(meta)
    return nc, meta, in_maps


_PENDING = {}


def kernel(**inputs):
    x = np.asarray(inputs["x"])
    n_nodes = x.shape[0]
    meta, in_maps = _prep(inputs)
    key = hash(
        (
            n_nodes,
            np.asarray(inputs["edge_src"]).tobytes(),
            np.asarray(inputs["edge_dst"]).tobytes(),
        )
    )
    _PENDING[key] = (meta, in_maps)
    nc, meta, _ = _get_compiled(key)

    from concourse.bass_utils import run_bass_kernel_spmd

    trace = bool(int(os.environ.get("KERNEL_TRACE", "0")))
    res = run_bass_kernel_spmd(
        nc, in_maps, core_ids=list(range(CORES)), trace=trace
    )
    kernel.last_results = res
    return _assemble(meta, res.results, n_nodes)

